# revision 21
# baseline (speedup 1.0000x reference)
"""Causal self-attention (B=4, T=2048, C=1024, H=16, D=64) on 8 NeuronCores.

Sharding: core = (batch b, head-group g) with b = core//2, g = core%2.
Each core computes its batch's attention for 8 heads (g picks heads 8g..8g+7)
plus the corresponding slice of the QKV/output projections (tensor parallel,
column/row split).  The output projection is row-parallel, so the full output
for batch b is the SUM of the two partial outputs of cores (2b, 2b+1); that
reduction is done on the host during the gather/unshard step.

Device kernel strategy (per core):
  - x^T materialized chunk-by-chunk via PE transposes (fp32 has no DMA
    transpose).
  - q^T, k^T computed as W^T @ x^T (so no transpose of activations needed);
    v computed in natural [T, D] layout as x @ Wv.  1/sqrt(D) is folded into
    Wq/bq on the host (exact: power of two).
  - scores are computed TRANSPOSED (k-position on partitions) so that the
    probs @ v contraction needs no transpose;  softmax runs without max
    subtraction (scores are bounded ~|2|, exp is safe) and the denominator
    comes free as a 65th "ones" column in the PV matmul.
  - causality by restricting matmul column ranges per k-tile + one 128x128
    triangle mask multiply per diagonal block.
  - all matmuls in float16 (1 cycle/row on PE + fast weight load; 10-bit
    mantissa inputs, fp32 PSUM accumulation).
"""

import os
import tempfile
from contextlib import ExitStack

import numpy as np

import concourse.bass as bass
import concourse.mybir as mybir
import concourse.tile as tile
from concourse import bacc
from concourse.bass_utils import run_bass_kernel_spmd
from concourse.masks import make_identity, make_upper_triangular

B, T_FULL, C, H, D = 4, 2048, 1024, 16, 64
HG = 2                # head-group (tensor-parallel) factor
GH = H // HG          # heads per core = 8
F = GH * D            # per-core projection width = 512
N_CORES = B * HG      # 8
FP = mybir.dt.float32
FPH = mybir.dt.float16


def _emit(nc: bass.Bass, T: int):
    CH = T // 512            # 512-row query chunks
    KO = C // 128            # contraction subtiles for C (8)
    FT = F // 128            # feature tiles (4)
    AF = mybir.ActivationFunctionType

    x = nc.dram_tensor("x", [T, C], FP, kind="ExternalInput").ap()
    wq = nc.dram_tensor("wq", [C, F], FPH, kind="ExternalInput").ap()
    wk = nc.dram_tensor("wk", [C, F], FPH, kind="ExternalInput").ap()
    wv = nc.dram_tensor("wv", [C, F], FPH, kind="ExternalInput").ap()
    bq = nc.dram_tensor("bq", [F], FP, kind="ExternalInput").ap()
    bk = nc.dram_tensor("bk", [F], FP, kind="ExternalInput").ap()
    bv = nc.dram_tensor("bv", [F], FP, kind="ExternalInput").ap()
    wo = nc.dram_tensor("wo", [F, C], FPH, kind="ExternalInput").ap()
    bo = nc.dram_tensor("bo", [C], FP, kind="ExternalInput").ap()
    y = nc.dram_tensor("y", [T, C], FP, kind="ExternalOutput").ap()

    with tile.TileContext(nc) as tc, ExitStack() as ctx:
        const = ctx.enter_context(tc.tile_pool(name="const", bufs=1))
        pers = ctx.enter_context(tc.tile_pool(name="pers", bufs=1))
        xp = ctx.enter_context(tc.tile_pool(name="xp", bufs=6))
        xtp = ctx.enter_context(tc.tile_pool(name="xtp", bufs=3))
        qtp = ctx.enter_context(tc.tile_pool(name="qtp", bufs=3))
        ptp = ctx.enter_context(tc.tile_pool(name="ptp", bufs=6))
        otp = ctx.enter_context(tc.tile_pool(name="otp", bufs=6))
        oup = ctx.enter_context(tc.tile_pool(name="oup", bufs=4))
        yp = ctx.enter_context(tc.tile_pool(name="yp", bufs=3))
        smp = ctx.enter_context(tc.tile_pool(name="smp", bufs=3))
        ps_big = ctx.enter_context(tc.tile_pool(name="ps_big", bufs=2, space="PSUM"))
        ps_pv = ctx.enter_context(tc.tile_pool(name="ps_pv", bufs=2, space="PSUM"))
        ps_sm = ctx.enter_context(tc.tile_pool(name="ps_sm", bufs=2, space="PSUM"))

        # ---- constants / weights ----------------------------------------
        ident = const.tile([128, 128], FP)
        make_identity(nc, ident[:])
        tri = const.tile([128, 128], FPH)       # tri[r, c] = 1.0 if c >= r else 0
        make_upper_triangular(nc, tri[:], val=1.0, diag=True)
        ones128 = const.tile([128, 128], FP)
        nc.gpsimd.memset(ones128[:], 1.0)

        wq_sb = const.tile([128, KO, F], FPH)
        wk_sb = const.tile([128, KO, F], FPH)
        wv_sb = const.tile([128, KO, F], FPH)
        wo_sb = const.tile([128, FT, C], FPH)
        bq_sb = const.tile([128, FT], FP)
        bk_sb = const.tile([128, FT], FP)
        bv_bc = const.tile([128, F], FP)
        bo_bc = const.tile([128, C], FP)

        def load_weights():
            # emitted AFTER chunk 0/1 x DMAs so the first transposes aren't
            # starved behind the weight bulk on the DMA queues
            nc.sync.dma_start(wq_sb[:], wq.rearrange("(ko p) f -> p ko f", p=128))
            nc.sync.dma_start(wk_sb[:], wk.rearrange("(ko p) f -> p ko f", p=128))
            nc.sync.dma_start(wv_sb[:], wv.rearrange("(ko p) f -> p ko f", p=128))
            nc.sync.dma_start(wo_sb[:], wo.rearrange("(ft p) c -> p ft c", p=128))
            nc.sync.dma_start(bq_sb[:], bq.rearrange("(ft p) -> p ft", p=128))
            nc.sync.dma_start(bk_sb[:], bk.rearrange("(ft p) -> p ft", p=128))
            nc.sync.dma_start(bv_bc[0:1, :], bv.rearrange("(o f) -> o f", o=1))
            nc.gpsimd.partition_broadcast(bv_bc[:], bv_bc[0:1, :])
            nc.sync.dma_start(bo_bc[0:1, :], bo.rearrange("(o c) -> o c", o=1))
            nc.gpsimd.partition_broadcast(bo_bc[:], bo_bc[0:1, :])

        # ---- persistent k^T / v (one tile per 512-chunk for precise deps)
        kt_c = []
        v_c = []
        for c in range(CH):
            kt_t = pers.tile([128, FT, 512], FPH, name=f"kt_{c}")
            kt_c.append(kt_t)
            v_t = pers.tile([128, 4, GH, D + 1], FPH, name=f"v_{c}")
            v_c.append(v_t)
            nc.vector.tensor_copy(
                v_t[:, :, :, D : D + 1],
                ones128[:, 0 : 4 * GH].rearrange("p (k h o) -> p k h o", k=4, o=1),
            )

        xt_all = [None] * CH
        qt_all = [None] * CH

        def ab_groups(c):
            """Emit chunk c's x DMAs now; return PE work-group closures."""
            xnat = [xp.tile([128, C], FP, tag="xnat", name=f"xnat_{c}_{i}") for i in range(4)]
            for s in range(4):
                nc.sync.dma_start(xnat[s][:], x[c * 512 + s * 128 : c * 512 + (s + 1) * 128, :])
            xt_sb = xtp.tile([128, KO, 512], FPH, tag="xt", name=f"xt_sb_{c}")
            xt_all[c] = xt_sb
            qt_sb = qtp.tile([128, FT, 512], FPH, tag="qt", name=f"qt_sb_{c}")
            qt_all[c] = qt_sb
            groups = []

            def transpose_group(kop):
                ps_t = ps_big.tile([128, 1024], FP, tag="ps_big", name=f"ps_t_{c}_{kop}")
                for u in range(2):
                    ko = kop * 2 + u
                    for s in range(4):
                        nc.tensor.transpose(
                            ps_t[:, u * 512 + s * 128 : u * 512 + (s + 1) * 128],
                            xnat[s][:, ko * 128 : (ko + 1) * 128],
                            ident[:],
                        )
                nc.vector.tensor_copy(xt_sb[:, kop * 2 : kop * 2 + 2, :], ps_t[:].rearrange("p (u t) -> p u t", u=2))

            def q_group(ft):
                ps_q = ps_sm.tile([128, 512], FP, tag="ps_sm", name=f"ps_q_{c}_{ft}")
                for ko in range(KO):
                    nc.tensor.matmul(
                        ps_q[:],
                        wq_sb[:, ko, ft * 128 : (ft + 1) * 128],
                        xt_sb[:, ko, :],
                        start=(ko == 0),
                        stop=(ko == KO - 1),
                    )
                nc.scalar.activation(qt_sb[:, ft, :], ps_q[:], AF.Identity, bias=bq_sb[:, ft : ft + 1])

            def k_group(ft):
                ps_k = ps_sm.tile([128, 512], FP, tag="ps_sm", name=f"ps_k_{c}_{ft}")
                for ko in range(KO):
                    nc.tensor.matmul(
                        ps_k[:],
                        wk_sb[:, ko, ft * 128 : (ft + 1) * 128],
                        xt_sb[:, ko, :],
                        start=(ko == 0),
                        stop=(ko == KO - 1),
                    )
                nc.scalar.activation(kt_c[c][:, ft, :], ps_k[:], AF.Identity, bias=bk_sb[:, ft : ft + 1])

            def v_group(s):
                ps_v = ps_sm.tile([128, 512], FP, tag="ps_sm", name=f"ps_v_{c}_{s}")
                for ko in range(KO):
                    nc.tensor.matmul(
                        ps_v[:],
                        xt_sb[:, ko, s * 128 : (s + 1) * 128],
                        wv_sb[:, ko, :],
                        start=(ko == 0),
                        stop=(ko == KO - 1),
                    )
                nc.vector.tensor_tensor(
                    v_c[c][:, s, :, 0:D],
                    ps_v[:].rearrange("p (h d) -> p h d", h=GH),
                    bv_bc[:].rearrange("p (h d) -> p h d", h=GH),
                    mybir.AluOpType.add,
                )

            for kop in range(KO // 2):
                groups.append(lambda kop=kop: transpose_group(kop))
            for ft in range(FT):
                groups.append(lambda ft=ft: q_group(ft))
            for ft in range(FT):
                groups.append(lambda ft=ft: k_group(ft))
            for s in range(4):
                groups.append(lambda s=s: v_group(s))
            return groups

        def emit_cd(c, pend):
            qt_sb = qt_all[c]
            # ---- C: attention, head pairs packed in PE row groups --------
            KT = 4 * (c + 1)
            ot = [otp.tile([128, 512], FPH, tag="ot", name=f"ot_{c}_{i}") for i in range(FT)]
            for hp in range(FT):
                pv_ab = []
                for half, po in ((0, 0), (1, 64)):
                    pv_t = ps_pv.tile([65, 512], FP, tag="ps_pv", name=f"pv_{c}_{hp}_{half}")
                    pv_ab.append(pv_t)
                for j in range(KT):
                    off = max(0, (j - 4 * c) * 128)
                    jc, js = j // 4, (j % 4) * 128
                    ps_s = ps_big.tile([128, 1024], FP, tag="ps_big", name=f"ps_s_{c}_{hp}_{j}")
                    pt_t = ptp.tile([128, 1024], FPH, tag="pt", name=f"pt_{c}_{hp}_{j}")
                    for half, po in ((0, 0), (1, 64)):
                        nc.tensor.matmul(
                            ps_s[:, half * 512 + off : (half + 1) * 512],
                            kt_c[jc][po : po + 64, hp, js : js + 128],
                            qt_sb[po : po + 64, hp, off:512],
                            start=True,
                            stop=True,
                            tile_position=(po, 0),
                        )
                    if off == 0:
                        nc.scalar.activation(pt_t[:], ps_s[:], AF.Exp)
                    else:
                        for half in range(2):
                            nc.scalar.activation(
                                pt_t[:, half * 512 + off : (half + 1) * 512],
                                ps_s[:, half * 512 + off : (half + 1) * 512],
                                AF.Exp,
                            )
                    if j >= 4 * c:
                        for half in range(2):
                            blk = slice(half * 512 + off, half * 512 + off + 128)
                            nc.vector.tensor_mul(pt_t[:, blk], pt_t[:, blk], tri[:])
                    for half in range(2):
                        nc.tensor.matmul(
                            pv_ab[half][:, off:512],
                            v_c[jc][:, j % 4, 2 * hp + half, :],
                            pt_t[:, half * 512 + off : (half + 1) * 512],
                            start=(j == 0),
                            stop=(j == KT - 1),
                        )
                    if j % 4 == 3 and pend:
                        pend.popleft()()
                for half, po in ((0, 0), (1, 64)):
                    ou_t = oup.tile([64, 512], FP, tag="ou", name=f"ou_{c}_{hp}_{half}")
                    nc.vector.tensor_copy(ou_t[:], pv_ab[half][0:64, :])
                    sum_t = smp.tile([1, 512], FP, tag="sum", name=f"sum_{c}_{hp}_{half}")
                    nc.vector.tensor_copy(sum_t[:], pv_ab[half][64:65, :])
                    r_t = smp.tile([1, 512], FP, tag="r", name=f"r_{c}_{hp}_{half}")
                    nc.vector.reciprocal_approx_fast(out=r_t[:], in_=sum_t[:])
                    rb_t = smp.tile([64, 512], FP, tag="rb", name=f"rb_{c}_{hp}_{half}")
                    nc.gpsimd.partition_broadcast(rb_t[:], r_t[:])
                    nc.vector.tensor_mul(ot[hp][po : po + 64, :], ou_t[:], rb_t[:])
                if pend:
                    pend.popleft()()

            # ---- D: output projection ------------------------------------
            for qs in range(4):
                for half in range(2):
                    ps_y = ps_sm.tile([128, 512], FP, tag="ps_sm", name=f"ps_y_{c}_{qs}_{half}")
                    for ft in range(FT):
                        nc.tensor.matmul(
                            ps_y[:],
                            ot[ft][:, qs * 128 : (qs + 1) * 128],
                            wo_sb[:, ft, half * 512 : (half + 1) * 512],
                            start=(ft == 0),
                            stop=(ft == FT - 1),
                        )
                    y_t = yp.tile([128, 512], FP, tag="y", name=f"y_{c}_{qs}_{half}")
                    nc.vector.tensor_tensor(
                        y_t[:], ps_y[:], bo_bc[:, half * 512 : (half + 1) * 512], mybir.AluOpType.add
                    )
                    nc.sync.dma_start(
                        y[c * 512 + qs * 128 : c * 512 + (qs + 1) * 128, half * 512 : (half + 1) * 512],
                        y_t[:],
                    )

        # software-pipelined by one chunk, with chunk c+2's PE work groups
        # interleaved INTO chunk c's (ACT-bound) attention loop so the PE
        # always has queued work during exp stretches.
        from collections import deque

        pend = deque()
        gq = ab_groups(0)
        load_weights()
        for g in gq:
            g()
        if CH > 1:
            for g in ab_groups(1):
                g()
        for c in range(CH):
            if c + 2 < CH:
                pend.extend(ab_groups(c + 2))
            emit_cd(c, pend)
            while pend:
                pend.popleft()()
_NC_CACHE: dict = {}


def build_nc(T: int = T_FULL):
    if T not in _NC_CACHE:
        nc = bacc.Bacc("TRN2", target_bir_lowering=False, debug=False, num_devices=N_CORES)
        _emit(nc, T)
        nc.compile()
        _NC_CACHE[T] = nc
    return _NC_CACHE[T]


def make_in_maps(x, Wqkv, bqkv, Wo, bo, T: int = T_FULL):
    """Shard full inputs into the 8 per-core input maps."""
    x = np.asarray(x, dtype=np.float32)
    Wqkv = np.asarray(Wqkv, dtype=np.float32)
    bqkv = np.asarray(bqkv, dtype=np.float32)
    Wo = np.asarray(Wo, dtype=np.float32)
    bo = np.asarray(bo, dtype=np.float32)
    zeros_c = np.zeros(C, dtype=np.float32)
    in_maps = []
    for core in range(N_CORES):
        b, g = core // HG, core % HG
        sl = slice(g * F, (g + 1) * F)
        in_maps.append(
            {
                "x": np.ascontiguousarray(x[b, :T]),
                "wq": (np.ascontiguousarray(Wqkv[:, sl]) * np.float32(0.125)).astype(np.float16),
                "wk": np.ascontiguousarray(Wqkv[:, C + g * F : C + (g + 1) * F]).astype(np.float16),
                "wv": np.ascontiguousarray(Wqkv[:, 2 * C + g * F : 2 * C + (g + 1) * F]).astype(np.float16),
                "bq": np.ascontiguousarray(bqkv[sl]) * np.float32(0.125),
                "bk": np.ascontiguousarray(bqkv[C + g * F : C + (g + 1) * F]),
                "bv": np.ascontiguousarray(bqkv[2 * C + g * F : 2 * C + (g + 1) * F]),
                "wo": np.ascontiguousarray(Wo[sl, :]).astype(np.float16),
                "bo": bo if g == 0 else zeros_c,
            }
        )
    return in_maps


def kernel(x, Wqkv, bqkv, Wo, bo):
    nc = build_nc(T_FULL)
    in_maps = make_in_maps(x, Wqkv, bqkv, Wo, bo)
    res = run_bass_kernel_spmd(nc, in_maps, core_ids=list(range(N_CORES)))
    out = np.empty((B, T_FULL, C), dtype=np.float32)
    for b in range(B):
        out[b] = res.results[HG * b]["y"] + res.results[HG * b + 1]["y"]
    return out


# revision 22
# speedup vs baseline: 1.0276x; 1.0276x over previous
"""Causal self-attention (B=4, T=2048, C=1024, H=16, D=64) on 8 NeuronCores.

Sharding: core = (batch b, head-group g) with b = core//2, g = core%2.
Each core computes its batch's attention for 8 heads (g picks heads 8g..8g+7)
plus the corresponding slice of the QKV/output projections (tensor parallel,
column/row split).  The output projection is row-parallel, so the full output
for batch b is the SUM of the two partial outputs of cores (2b, 2b+1); that
reduction is done on the host during the gather/unshard step.

Device kernel strategy (per core):
  - x^T materialized chunk-by-chunk via PE transposes (fp32 has no DMA
    transpose).
  - q^T, k^T computed as W^T @ x^T (so no transpose of activations needed);
    v computed in natural [T, D] layout as x @ Wv.  1/sqrt(D) is folded into
    Wq/bq on the host (exact: power of two).
  - scores are computed TRANSPOSED (k-position on partitions) so that the
    probs @ v contraction needs no transpose;  softmax runs without max
    subtraction (scores are bounded ~|2|, exp is safe) and the denominator
    comes free as a 65th "ones" column in the PV matmul.
  - causality by restricting matmul column ranges per k-tile + one 128x128
    triangle mask multiply per diagonal block.
  - all matmuls in float16 (1 cycle/row on PE + fast weight load; 10-bit
    mantissa inputs, fp32 PSUM accumulation).
"""

import os
import tempfile
from contextlib import ExitStack

import numpy as np

import concourse.bass as bass
import concourse.mybir as mybir
import concourse.tile as tile
from concourse import bacc
from concourse.bass_utils import run_bass_kernel_spmd
from concourse.masks import make_identity, make_upper_triangular

B, T_FULL, C, H, D = 4, 2048, 1024, 16, 64
HG = 2                # head-group (tensor-parallel) factor
GH = H // HG          # heads per core = 8
F = GH * D            # per-core projection width = 512
N_CORES = B * HG      # 8
FP = mybir.dt.float32
FPH = mybir.dt.float16


def _emit(nc: bass.Bass, T: int):
    CH = T // 512            # 512-row query chunks
    KO = C // 128            # contraction subtiles for C (8)
    FT = F // 128            # feature tiles (4)
    AF = mybir.ActivationFunctionType

    x = nc.dram_tensor("x", [T, C], FP, kind="ExternalInput").ap()
    wq = nc.dram_tensor("wq", [C, F], FPH, kind="ExternalInput").ap()
    wk = nc.dram_tensor("wk", [C, F], FPH, kind="ExternalInput").ap()
    wv = nc.dram_tensor("wv", [C, F], FPH, kind="ExternalInput").ap()
    bq = nc.dram_tensor("bq", [F], FP, kind="ExternalInput").ap()
    bk = nc.dram_tensor("bk", [F], FP, kind="ExternalInput").ap()
    bv = nc.dram_tensor("bv", [F], FP, kind="ExternalInput").ap()
    wo = nc.dram_tensor("wo", [F, C], FPH, kind="ExternalInput").ap()
    bo = nc.dram_tensor("bo", [C], FP, kind="ExternalInput").ap()
    y = nc.dram_tensor("y", [T, C], FP, kind="ExternalOutput").ap()

    with tile.TileContext(nc) as tc, ExitStack() as ctx:
        const = ctx.enter_context(tc.tile_pool(name="const", bufs=1))
        pers = ctx.enter_context(tc.tile_pool(name="pers", bufs=1))
        xp = ctx.enter_context(tc.tile_pool(name="xp", bufs=6))
        xtp = ctx.enter_context(tc.tile_pool(name="xtp", bufs=3))
        qtp = ctx.enter_context(tc.tile_pool(name="qtp", bufs=3))
        ptp = ctx.enter_context(tc.tile_pool(name="ptp", bufs=6))
        otp = ctx.enter_context(tc.tile_pool(name="otp", bufs=6))
        oup = ctx.enter_context(tc.tile_pool(name="oup", bufs=4))
        yp = ctx.enter_context(tc.tile_pool(name="yp", bufs=3))
        smp = ctx.enter_context(tc.tile_pool(name="smp", bufs=3))
        ps_big = ctx.enter_context(tc.tile_pool(name="ps_big", bufs=2, space="PSUM"))
        ps_pv = ctx.enter_context(tc.tile_pool(name="ps_pv", bufs=2, space="PSUM"))
        ps_sm = ctx.enter_context(tc.tile_pool(name="ps_sm", bufs=2, space="PSUM"))

        # ---- constants / weights ----------------------------------------
        ident = const.tile([128, 128], FP)
        make_identity(nc, ident[:])
        tri = const.tile([128, 128], FPH)       # tri[r, c] = 1.0 if c >= r else 0
        make_upper_triangular(nc, tri[:], val=1.0, diag=True)
        ones128 = const.tile([128, 128], FP)
        nc.gpsimd.memset(ones128[:], 1.0)

        wq_sb = const.tile([128, KO, F], FPH)
        wk_sb = const.tile([128, KO, F], FPH)
        wv_sb = const.tile([128, KO, F], FPH)
        wo_sb = const.tile([128, FT, C], FPH)
        bq_sb = const.tile([128, FT], FP)
        bk_sb = const.tile([128, FT], FP)
        bv_bc = const.tile([128, F], FP)
        bo_bc = const.tile([128, C], FP)

        def load_weights():
            # emitted AFTER chunk 0/1 x DMAs so the first transposes aren't
            # starved behind the weight bulk on the DMA queues
            nc.sync.dma_start(wq_sb[:], wq.rearrange("(ko p) f -> p ko f", p=128))
            nc.sync.dma_start(wk_sb[:], wk.rearrange("(ko p) f -> p ko f", p=128))
            nc.sync.dma_start(wv_sb[:], wv.rearrange("(ko p) f -> p ko f", p=128))
            nc.sync.dma_start(wo_sb[:], wo.rearrange("(ft p) c -> p ft c", p=128))
            nc.sync.dma_start(bq_sb[:], bq.rearrange("(ft p) -> p ft", p=128))
            nc.sync.dma_start(bk_sb[:], bk.rearrange("(ft p) -> p ft", p=128))
            nc.sync.dma_start(bv_bc[0:1, :], bv.rearrange("(o f) -> o f", o=1))
            nc.gpsimd.partition_broadcast(bv_bc[:], bv_bc[0:1, :])
            nc.sync.dma_start(bo_bc[0:1, :], bo.rearrange("(o c) -> o c", o=1))
            nc.gpsimd.partition_broadcast(bo_bc[:], bo_bc[0:1, :])

        # ---- persistent k^T / v (one tile per 512-chunk for precise deps)
        kt_c = []
        v_c = []
        for c in range(CH):
            kt_t = pers.tile([128, FT, 512], FPH, name=f"kt_{c}")
            kt_c.append(kt_t)
            v_t = pers.tile([128, 4, GH, D + 1], FPH, name=f"v_{c}")
            v_c.append(v_t)
            nc.vector.tensor_copy(
                v_t[:, :, :, D : D + 1],
                ones128[:, 0 : 4 * GH].rearrange("p (k h o) -> p k h o", k=4, o=1),
            )

        xt_all = [None] * CH
        qt_all = [None] * CH

        def ab_groups(c):
            """Emit chunk c's x DMAs now; return PE work-group closures."""
            xnat = [xp.tile([128, C], FP, tag="xnat", name=f"xnat_{c}_{i}") for i in range(4)]
            for s in range(4):
                nc.sync.dma_start(xnat[s][:], x[c * 512 + s * 128 : c * 512 + (s + 1) * 128, :])
            xt_sb = xtp.tile([128, KO, 512], FPH, tag="xt", name=f"xt_sb_{c}")
            xt_all[c] = xt_sb
            qt_sb = qtp.tile([128, FT, 512], FPH, tag="qt", name=f"qt_sb_{c}")
            qt_all[c] = qt_sb
            groups = []

            def transpose_group(kop):
                ps_t = ps_big.tile([128, 1024], FP, tag="ps_big", name=f"ps_t_{c}_{kop}")
                for u in range(2):
                    ko = kop * 2 + u
                    for s in range(4):
                        nc.tensor.transpose(
                            ps_t[:, u * 512 + s * 128 : u * 512 + (s + 1) * 128],
                            xnat[s][:, ko * 128 : (ko + 1) * 128],
                            ident[:],
                        )
                nc.vector.tensor_copy(xt_sb[:, kop * 2 : kop * 2 + 2, :], ps_t[:].rearrange("p (u t) -> p u t", u=2))

            def q_group(ft):
                ps_q = ps_sm.tile([128, 512], FP, tag="ps_sm", name=f"ps_q_{c}_{ft}")
                for ko in range(KO):
                    nc.tensor.matmul(
                        ps_q[:],
                        wq_sb[:, ko, ft * 128 : (ft + 1) * 128],
                        xt_sb[:, ko, :],
                        start=(ko == 0),
                        stop=(ko == KO - 1),
                    )
                nc.vector.tensor_scalar_add(qt_sb[:, ft, :], ps_q[:], bq_sb[:, ft : ft + 1])

            def k_group(ft):
                ps_k = ps_sm.tile([128, 512], FP, tag="ps_sm", name=f"ps_k_{c}_{ft}")
                for ko in range(KO):
                    nc.tensor.matmul(
                        ps_k[:],
                        wk_sb[:, ko, ft * 128 : (ft + 1) * 128],
                        xt_sb[:, ko, :],
                        start=(ko == 0),
                        stop=(ko == KO - 1),
                    )
                nc.vector.tensor_scalar_add(kt_c[c][:, ft, :], ps_k[:], bk_sb[:, ft : ft + 1])

            def v_group(s):
                ps_v = ps_sm.tile([128, 512], FP, tag="ps_sm", name=f"ps_v_{c}_{s}")
                for ko in range(KO):
                    nc.tensor.matmul(
                        ps_v[:],
                        xt_sb[:, ko, s * 128 : (s + 1) * 128],
                        wv_sb[:, ko, :],
                        start=(ko == 0),
                        stop=(ko == KO - 1),
                    )
                nc.vector.tensor_tensor(
                    v_c[c][:, s, :, 0:D],
                    ps_v[:].rearrange("p (h d) -> p h d", h=GH),
                    bv_bc[:].rearrange("p (h d) -> p h d", h=GH),
                    mybir.AluOpType.add,
                )

            for kop in range(KO // 2):
                groups.append(lambda kop=kop: transpose_group(kop))
            for ft in range(FT):
                groups.append(lambda ft=ft: q_group(ft))
            for ft in range(FT):
                groups.append(lambda ft=ft: k_group(ft))
            for s in range(4):
                groups.append(lambda s=s: v_group(s))
            return groups

        def emit_cd(c, pend):
            qt_sb = qt_all[c]
            # ---- C: attention, head pairs packed in PE row groups --------
            KT = 4 * (c + 1)
            ot = [otp.tile([128, 512], FPH, tag="ot", name=f"ot_{c}_{i}") for i in range(FT)]
            for hp in range(FT):
                pv_ab = []
                for half, po in ((0, 0), (1, 64)):
                    pv_t = ps_pv.tile([65, 512], FP, tag="ps_pv", name=f"pv_{c}_{hp}_{half}")
                    pv_ab.append(pv_t)
                for j in range(KT):
                    off = max(0, (j - 4 * c) * 128)
                    jc, js = j // 4, (j % 4) * 128
                    ps_s = ps_big.tile([128, 1024], FP, tag="ps_big", name=f"ps_s_{c}_{hp}_{j}")
                    pt_t = ptp.tile([128, 1024], FPH, tag="pt", name=f"pt_{c}_{hp}_{j}")
                    for half, po in ((0, 0), (1, 64)):
                        nc.tensor.matmul(
                            ps_s[:, half * 512 + off : (half + 1) * 512],
                            kt_c[jc][po : po + 64, hp, js : js + 128],
                            qt_sb[po : po + 64, hp, off:512],
                            start=True,
                            stop=True,
                            tile_position=(po, 0),
                        )
                    if off == 0:
                        nc.scalar.activation(pt_t[:], ps_s[:], AF.Exp)
                    else:
                        for half in range(2):
                            nc.scalar.activation(
                                pt_t[:, half * 512 + off : (half + 1) * 512],
                                ps_s[:, half * 512 + off : (half + 1) * 512],
                                AF.Exp,
                            )
                    if j >= 4 * c:
                        for half in range(2):
                            blk = slice(half * 512 + off, half * 512 + off + 128)
                            nc.vector.tensor_mul(pt_t[:, blk], pt_t[:, blk], tri[:])
                    for half in range(2):
                        nc.tensor.matmul(
                            pv_ab[half][:, off:512],
                            v_c[jc][:, j % 4, 2 * hp + half, :],
                            pt_t[:, half * 512 + off : (half + 1) * 512],
                            start=(j == 0),
                            stop=(j == KT - 1),
                        )
                    if j % 4 == 3 and pend:
                        pend.popleft()()
                for half, po in ((0, 0), (1, 64)):
                    ou_t = oup.tile([64, 512], FP, tag="ou", name=f"ou_{c}_{hp}_{half}")
                    nc.vector.tensor_copy(ou_t[:], pv_ab[half][0:64, :])
                    sum_t = smp.tile([1, 512], FP, tag="sum", name=f"sum_{c}_{hp}_{half}")
                    nc.vector.tensor_copy(sum_t[:], pv_ab[half][64:65, :])
                    r_t = smp.tile([1, 512], FP, tag="r", name=f"r_{c}_{hp}_{half}")
                    nc.vector.reciprocal_approx_fast(out=r_t[:], in_=sum_t[:])
                    rb_t = smp.tile([64, 512], FP, tag="rb", name=f"rb_{c}_{hp}_{half}")
                    nc.gpsimd.partition_broadcast(rb_t[:], r_t[:])
                    nc.vector.tensor_mul(ot[hp][po : po + 64, :], ou_t[:], rb_t[:])
                if pend:
                    pend.popleft()()

            # ---- D: output projection ------------------------------------
            for qs in range(4):
                for half in range(2):
                    ps_y = ps_sm.tile([128, 512], FP, tag="ps_sm", name=f"ps_y_{c}_{qs}_{half}")
                    for ft in range(FT):
                        nc.tensor.matmul(
                            ps_y[:],
                            ot[ft][:, qs * 128 : (qs + 1) * 128],
                            wo_sb[:, ft, half * 512 : (half + 1) * 512],
                            start=(ft == 0),
                            stop=(ft == FT - 1),
                        )
                    y_t = yp.tile([128, 512], FP, tag="y", name=f"y_{c}_{qs}_{half}")
                    nc.vector.tensor_tensor(
                        y_t[:], ps_y[:], bo_bc[:, half * 512 : (half + 1) * 512], mybir.AluOpType.add
                    )
                    nc.sync.dma_start(
                        y[c * 512 + qs * 128 : c * 512 + (qs + 1) * 128, half * 512 : (half + 1) * 512],
                        y_t[:],
                    )

        # software-pipelined by one chunk, with chunk c+2's PE work groups
        # interleaved INTO chunk c's (ACT-bound) attention loop so the PE
        # always has queued work during exp stretches.
        from collections import deque

        pend = deque()
        gq = ab_groups(0)
        load_weights()
        for g in gq:
            g()
        if CH > 1:
            for g in ab_groups(1):
                g()
        for c in range(CH):
            if c + 2 < CH:
                pend.extend(ab_groups(c + 2))
            emit_cd(c, pend)
            while pend:
                pend.popleft()()
_NC_CACHE: dict = {}


def build_nc(T: int = T_FULL):
    if T not in _NC_CACHE:
        nc = bacc.Bacc("TRN2", target_bir_lowering=False, debug=False, num_devices=N_CORES)
        _emit(nc, T)
        nc.compile()
        _NC_CACHE[T] = nc
    return _NC_CACHE[T]


def make_in_maps(x, Wqkv, bqkv, Wo, bo, T: int = T_FULL):
    """Shard full inputs into the 8 per-core input maps."""
    x = np.asarray(x, dtype=np.float32)
    Wqkv = np.asarray(Wqkv, dtype=np.float32)
    bqkv = np.asarray(bqkv, dtype=np.float32)
    Wo = np.asarray(Wo, dtype=np.float32)
    bo = np.asarray(bo, dtype=np.float32)
    zeros_c = np.zeros(C, dtype=np.float32)
    in_maps = []
    for core in range(N_CORES):
        b, g = core // HG, core % HG
        sl = slice(g * F, (g + 1) * F)
        in_maps.append(
            {
                "x": np.ascontiguousarray(x[b, :T]),
                "wq": (np.ascontiguousarray(Wqkv[:, sl]) * np.float32(0.125)).astype(np.float16),
                "wk": np.ascontiguousarray(Wqkv[:, C + g * F : C + (g + 1) * F]).astype(np.float16),
                "wv": np.ascontiguousarray(Wqkv[:, 2 * C + g * F : 2 * C + (g + 1) * F]).astype(np.float16),
                "bq": np.ascontiguousarray(bqkv[sl]) * np.float32(0.125),
                "bk": np.ascontiguousarray(bqkv[C + g * F : C + (g + 1) * F]),
                "bv": np.ascontiguousarray(bqkv[2 * C + g * F : 2 * C + (g + 1) * F]),
                "wo": np.ascontiguousarray(Wo[sl, :]).astype(np.float16),
                "bo": bo if g == 0 else zeros_c,
            }
        )
    return in_maps


def kernel(x, Wqkv, bqkv, Wo, bo):
    nc = build_nc(T_FULL)
    in_maps = make_in_maps(x, Wqkv, bqkv, Wo, bo)
    res = run_bass_kernel_spmd(nc, in_maps, core_ids=list(range(N_CORES)))
    out = np.empty((B, T_FULL, C), dtype=np.float32)
    for b in range(B):
        out[b] = res.results[HG * b]["y"] + res.results[HG * b + 1]["y"]
    return out
